# revision 9
# baseline (speedup 1.0000x reference)
"""Trainium2 Bass kernel for nn_MeanDegConv (gnn_message_passing) on 8 NeuronCores.

Round-based design: incidences are laid out as (window, round, slot) grids so
segment sums become PSUM-accumulating identity/diag matmuls (no per-tile
one-hot builds on the vector engine). Gather tables are bf16 (halved DMA
bytes) and gather descriptor generation rotates across SWDGE queues.

Self-contained: imports the Bass/Tile stack from /opt/trn_rl_repo (part of the
container environment) and hardcodes all shapes/sharding for the problem.
"""
import sys
for _p in ('/opt/trn_rl_repo',):
    if _p not in sys.path:
        sys.path.insert(0, _p)

import numpy as np

import concourse.bass as bass
import concourse.mybir as mybir
import concourse.tile as tile
import concourse.bacc as bacc
from concourse.bass_utils import run_bass_kernel_spmd

N, E, NNZ, D = 50000, 10000, 1000000, 128
C = 8
EPC, VPC = E // C, N // C          # 1250 edges, 6250 vertices per core
NWIN_E = (EPC + 127) // 128        # 10
NWIN_V = (VPC + 127) // 128        # 49
EP = NWIN_E * 128                  # 1280 padded edge slots per core
VP = NWIN_V * 128                  # 6272 padded vertex slots per core
CHUNK = 4096                       # gather indices per dma_gather call
TPC = CHUNK // 128                 # tiles per chunk
NQ = 4                             # SWDGE queues to rotate desc-gen across

SPLA = 32767                       # XA covers vertices [0, 32767); zero row at 32767
XB_BASE = N - SPLA                 # 17233; XB covers [17233, 50000); zero row at 32767
XTAB_ROWS = SPLA + 1               # 32768 rows per split table
XE_ROWS = C * EP                   # 10240 real xe rows
XE_ZERO = XE_ROWS                  # zero row index in xe_tab
XE_TAB_ROWS = XE_ROWS + 16         # padded alloc

F32 = mybir.dt.float32
BF16 = mybir.dt.bfloat16
I16 = mybir.dt.int16
BFNP = mybir.dt.np(BF16)


def _pack_idx16(idx32: np.ndarray) -> np.ndarray:
    """[L] int32 -> [128, L/16] int16 in the dma_gather wrap layout."""
    L = len(idx32)
    assert L % 16 == 0
    a = idx32.astype(np.int16).reshape(L // 16, 16).T  # [16, L/16]
    return np.ascontiguousarray(np.tile(a, (8, 1)))    # [128, L/16]


def _padlen(L):
    return ((L + CHUNK - 1) // CHUNK) * CHUNK


def prepare(inputs):
    X = np.asarray(inputs["X"], np.float32)
    X0 = np.asarray(inputs["X0"], np.float32)
    v = np.asarray(inputs["vertex"]).astype(np.int64)
    e = np.asarray(inputs["edges"]).astype(np.int64)
    W1_w = np.asarray(inputs["W1_w"], np.float32); W1_b = np.asarray(inputs["W1_b"], np.float32)
    W2_w = np.asarray(inputs["W2_w"], np.float32); W2_b = np.asarray(inputs["W2_b"], np.float32)
    W3_w1 = np.asarray(inputs["W3_w1"], np.float32); W3_b1 = np.asarray(inputs["W3_b1"], np.float32)
    W3_w2 = np.asarray(inputs["W3_w2"], np.float32); W3_b2 = np.asarray(inputs["W3_b2"], np.float32)

    deg_e = np.bincount(e, minlength=E)
    deg_v = np.bincount(v, minlength=N)

    # ---- folded weight matrices (float64 for accuracy, cast at the end)
    W2a = W2_w[:D].astype(np.float64); W2b1 = W2_w[D:2*D].astype(np.float64)
    w2b_log = W2_w[2*D].astype(np.float64)
    R1 = W3_w1[:D].astype(np.float64); R2 = W3_w1[D:2*D].astype(np.float64)
    R3 = W3_w1[2*D:3*D].astype(np.float64); r4 = W3_w1[3*D].astype(np.float64)
    W2bR = W2b1 @ R1
    K1 = (W1_w.astype(np.float64) @ W2bR).astype(np.float32)
    k2 = (w2b_log @ R1).astype(np.float32)
    c1 = (W1_b.astype(np.float64) @ W2bR).astype(np.float32)
    MX = (W2a @ R1 + R2).astype(np.float32)
    MX0 = R3.astype(np.float32)
    c0 = (W2_b.astype(np.float64) @ R1 + W3_b1).astype(np.float32)

    # ---- permutations: sort by degree desc, deal round-robin to cores
    eperm = np.argsort(-deg_e, kind="stable")
    e_core = np.empty(E, np.int64); e_pos = np.empty(E, np.int64)
    e_core[eperm] = np.arange(E) % C
    e_pos[eperm] = np.arange(E) // C
    vperm = np.argsort(-deg_v, kind="stable")
    v_core = np.empty(N, np.int64); v_pos = np.empty(N, np.int64)
    v_core[vperm] = np.arange(N) % C
    v_pos[vperm] = np.arange(N) // C

    # ---- stage 1: A/B balanced split per edge
    cls = np.where(v < XB_BASE, 0, np.where(v >= SPLA, 2, 1))
    nAf = np.bincount(e[cls == 0], minlength=E)
    nBf = np.bincount(e[cls == 2], minlength=E)
    cntA = np.clip((deg_e + 1) // 2, nAf, deg_e - nBf)

    cA = np.zeros((C, EP), np.int64); cB = np.zeros((C, EP), np.int64)
    cA[e_core, e_pos] = cntA
    cB[e_core, e_pos] = deg_e - cntA
    RA = cA.reshape(C, NWIN_E, 128).max(axis=(0, 2))
    RB = cB.reshape(C, NWIN_E, 128).max(axis=(0, 2))
    LA = int(RA.sum()) * 128
    LB = int(RB.sum()) * 128

    # order incidences by (edge, class): forced-A, middles, forced-B
    oinc = np.argsort(e * 4 + cls, kind="stable")
    e_s = e[oinc]; v_s = v[oinc]
    starts = np.searchsorted(e_s, np.arange(E))
    rank = np.arange(NNZ) - starts[e_s]
    isA = rank < cntA[e_s]
    ecore1 = e_core[e_s]; epos1 = e_pos[e_s]
    w1 = epos1 // 128; s1 = epos1 % 128
    offA = np.zeros(NWIN_E, np.int64); offA[1:] = np.cumsum(RA)[:-1]
    offB = np.zeros(NWIN_E, np.int64); offB[1:] = np.cumsum(RB)[:-1]
    posA = (offA[w1] + rank) * 128 + s1
    posB = (offB[w1] + (rank - cntA[e_s])) * 128 + s1

    # ---- stage 2 rounds
    cV = np.zeros((C, VP), np.int64)
    cV[v_core, v_pos] = deg_v
    R2r = cV.reshape(C, NWIN_V, 128).max(axis=(0, 2))
    L2 = int(R2r.sum()) * 128

    rowid_of_e = e_core * EP + e_pos
    o2 = np.argsort(v, kind="stable")
    v_s2 = v[o2]; e_s2 = e[o2]
    starts2 = np.searchsorted(v_s2, np.arange(N))
    rank2 = np.arange(NNZ) - starts2[v_s2]
    vcore2 = v_core[v_s2]; vpos2 = v_pos[v_s2]
    w2 = vpos2 // 128; s2w = vpos2 % 128
    off2 = np.zeros(NWIN_V, np.int64); off2[1:] = np.cumsum(R2r)[:-1]
    pos2 = (off2[w2] + rank2) * 128 + s2w
    val2 = rowid_of_e[e_s2]

    sched = {"RA": [int(x) for x in RA], "RB": [int(x) for x in RB],
             "R2": [int(x) for x in R2r],
             "LA": _padlen(LA), "LB": _padlen(LB), "L2": _padlen(L2)}

    # ---- shared consts
    Xb = X.astype(BFNP)
    XA = np.zeros((XTAB_ROWS, D), BFNP); XA[:SPLA] = Xb[:SPLA]
    XBt = np.zeros((XTAB_ROWS, D), BFNP); XBt[:N - XB_BASE] = Xb[XB_BASE:]
    deg_ef = deg_e.astype(np.float32); log_deg_e = np.log(deg_ef)
    deg_vf = deg_v.astype(np.float32); log_deg_v = np.log(deg_vf)

    consts = {
        "XA": XA, "XB": XBt,
        "iota": np.tile(np.arange(128, dtype=np.float32), (128, 1)).astype(BFNP),
        "iotacol": np.arange(128, dtype=np.float32).reshape(128, 1),
        "K1": K1.astype(BFNP),
        "K2": np.ascontiguousarray(np.stack([k2, c1])),
        "MX": MX, "MX0": MX0,
        "RC2": np.ascontiguousarray(np.stack([r4.astype(np.float32), c0])),
        "W3w2": W3_w2.astype(BFNP),
        "b2row": W3_b2.reshape(1, D),
        "ones1": np.ones((1, 128), np.float32),
    }

    edge_at = np.full((C, EP), -1, np.int64)
    edge_at[e_core, e_pos] = np.arange(E)
    vert_at = np.full((C, VP), -1, np.int64)
    vert_at[v_core, v_pos] = np.arange(N)

    in_maps = []
    for c in range(C):
        mA = (ecore1 == c) & isA
        mB = (ecore1 == c) & (~isA)
        sA = np.full(sched["LA"], SPLA, np.int32)
        sA[posA[mA]] = v_s[mA]
        sB = np.full(sched["LB"], SPLA, np.int32)
        sB[posB[mB]] = v_s[mB] - XB_BASE

        m2 = vcore2 == c
        s2 = np.full(sched["L2"], XE_ZERO, np.int32)
        s2[pos2[m2]] = val2[m2]

        de = np.ones(EP, np.float32); le = np.zeros(EP, np.float32)
        msk = edge_at[c] >= 0
        de[msk] = deg_ef[edge_at[c][msk]]
        le[msk] = log_deg_e[edge_at[c][msk]]
        auxe = np.ascontiguousarray(np.stack([de * le, de]))
        invde_col = np.ascontiguousarray((1.0 / de).reshape(NWIN_E, 128).T)

        dv = np.ones(VP, np.float32); lv = np.zeros(VP, np.float32)
        vm = vert_at[c] >= 0
        dv[vm] = deg_vf[vert_at[c][vm]]
        lv[vm] = log_deg_v[vert_at[c][vm]]
        auxv = np.ascontiguousarray(np.stack([lv, np.ones(VP, np.float32)]))
        invdv_col = np.ascontiguousarray((1.0 / dv).reshape(NWIN_V, 128).T)

        Xp = np.zeros((VP, D), np.float32); X0p = np.zeros((VP, D), np.float32)
        Xp[vm] = X[vert_at[c][vm]]
        X0p[vm] = X0[vert_at[c][vm]]

        m = dict(consts)
        m.update({
            "idxA": _pack_idx16(sA), "idxB": _pack_idx16(sB),
            "idx2": _pack_idx16(s2),
            "auxe": auxe, "invde_col": invde_col,
            "auxv": auxv, "invdv_col": invdv_col,
            "XT": np.ascontiguousarray(Xp.T), "X0T": np.ascontiguousarray(X0p.T),
        })
        in_maps.append(m)
    unperm = {"v_core": v_core, "v_pos": v_pos}
    return in_maps, sched, unperm


def build(in_map0, sched, nq=NQ):
    RA, RB, R2 = sched["RA"], sched["RB"], sched["R2"]
    nc = bacc.Bacc(None, num_swdge_queues=nq, dynamic_dma_scratch_size=32768)

    def param(name, dt=F32):
        arr = in_map0[name]
        return nc.declare_dram_parameter(name, list(arr.shape), dt, isOutput=False)

    XA_d = param("XA", BF16); XB_d = param("XB", BF16)
    iota_d = param("iota", BF16); iotacol_d = param("iotacol")
    K1_d = param("K1", BF16); K2_d = param("K2")
    MX_d = param("MX"); MX0_d = param("MX0"); RC2_d = param("RC2")
    W3w2_d = param("W3w2", BF16); b2row_d = param("b2row"); ones1_d = param("ones1")
    idxA_d = param("idxA", I16); idxB_d = param("idxB", I16); idx2_d = param("idx2", I16)
    auxe_d = param("auxe"); invde_d = param("invde_col")
    auxv_d = param("auxv"); invdv_d = param("invdv_col")
    XT_d = param("XT"); X0T_d = param("X0T")
    out_d = nc.declare_dram_parameter("out", [VP, D], F32, isOutput=True)

    # Rotate desc-gen across SWDGE queues 1..3: their Q7 core-pairs generate
    # descriptors off the Pool engine's critical path (queue 0 blocks ~32us).
    qrot = [1, 2, 3] if nq == 4 else list(range(nq))
    qctr = [0]

    def next_q():
        q = qrot[qctr[0] % len(qrot)]
        qctr[0] += 1
        return q

    with tile.TileContext(nc) as tc:
        with (
            tc.tile_pool(name="const", bufs=1) as cp,
            tc.tile_pool(name="stream", bufs=1) as sp,
            tc.tile_pool(name="g", bufs=6) as gp,
            tc.tile_pool(name="work", bufs=3) as wp,
            tc.tile_pool(name="psA", bufs=2, space="PSUM") as psA,
            tc.tile_pool(name="psT", bufs=2, space="PSUM") as psT,
            tc.tile_pool(name="psF", bufs=2, space="PSUM") as psF,
            tc.tile_pool(name="dram", bufs=1, space="DRAM") as dp,
        ):
            def load(pool, dram_ap, name, dt=F32, eng=None):
                t = pool.tile(list(dram_ap.shape), dt, name=name, tag=name)
                (eng or nc.sync).dma_start(t[:], dram_ap[:])
                return t

            idxA_t = load(sp, idxA_d, "idxA", I16)
            idxB_t = load(sp, idxB_d, "idxB", I16)
            idx2_t = load(sp, idx2_d, "idx2", I16)
            iota_t = load(cp, iota_d, "iota", BF16)
            iotacol_t = load(cp, iotacol_d, "iotacol")
            K1_t = load(cp, K1_d, "K1", BF16); K2_t = load(cp, K2_d, "K2")
            MX_t = load(cp, MX_d, "MX"); MX0_t = load(cp, MX0_d, "MX0")
            RC2_t = load(cp, RC2_d, "RC2")
            W3w2_t = load(cp, W3w2_d, "W3w2", BF16)
            b2row_t = load(cp, b2row_d, "b2row"); ones1_t = load(cp, ones1_d, "ones1")
            auxe_t = load(cp, auxe_d, "auxe"); invde_t = load(cp, invde_d, "invde")
            auxv_t = load(cp, auxv_d, "auxv"); invdv_t = load(cp, invdv_d, "invdv")
            XT_t = load(cp, XT_d, "XT", eng=nc.scalar)
            X0T_t = load(cp, X0T_d, "X0T", eng=nc.scalar)

            # identity (bf16): Id[s, j] = (iota[s, j] == s)
            Id_t = cp.tile([128, 128], BF16, name="Id", tag="Id")
            nc.vector.tensor_scalar(
                out=Id_t[:], in0=iota_t[:], scalar1=iotacol_t[:, 0:1],
                scalar2=None, op0=mybir.AluOpType.is_equal)
            # per-vertex-window diag(1/deg_v) bf16
            diag_t = cp.tile([128, NWIN_V, 128], BF16, name="diag", tag="diag")
            for w in range(NWIN_V):
                nc.vector.tensor_scalar(
                    out=diag_t[:, w, :], in0=iota_t[:], scalar1=iotacol_t[:, 0:1],
                    scalar2=invdv_t[:, w:w + 1], op0=mybir.AluOpType.is_equal,
                    op1=mybir.AluOpType.mult)

            H = (NWIN_E // 2) * 128
            xe_lo = dp.tile([H, D], BF16)
            xe_hi = dp.tile([EP - H, D], BF16)
            xe_all_lo = dp.tile([C * (NWIN_E // 2) * 128, D], BF16, addr_space="Shared")
            xe_all_hi = dp.tile([C * (EP - (NWIN_E // 2) * 128), D], BF16, addr_space="Shared")
            xe_tab = dp.tile([XE_TAB_ROWS, D], BF16)
            zrow = wp.tile([16, D], BF16, tag="zrow", name="zrow")
            nc.vector.memset(zrow[:], 0.0)
            nc.sync.dma_start(xe_tab[XE_ZERO:XE_ZERO + 16, :], zrow[:])

            # ---- lazy chunked gathers: one shared SBUF ring, issue on demand
            chunks = {}

            def get_tile(stream, pos, idx_t, in_ap):
                lst = chunks.setdefault(stream, [])
                ci = pos // CHUNK
                while len(lst) <= ci:
                    k = len(lst)
                    g = gp.tile([128, TPC, D], BF16, tag="g", name=f"g{stream}{k}")
                    nc.gpsimd.dma_gather(
                        out_ap=g[:], in_ap=in_ap,
                        idxs_ap=idx_t[:, k * (CHUNK // 16):(k + 1) * (CHUNK // 16)],
                        num_idxs=CHUNK, num_idxs_reg=CHUNK,
                        single_packet=False, elem_size=D, queue_num=next_q())
                    lst.append(g)
                return lst[ci][:, (pos % CHUNK) // 128, :]

            # ============ stage 1 ============
            pA = [0]; pB = [0]
            for w in range(NWIN_E):
                ra, rb = RA[w], RB[w]
                ps = psA.tile([128, 128], F32, tag="acc", name=f"psS{w}")
                for r in range(ra):
                    t = get_tile("A", pA[0], idxA_t, XA_d[:]); pA[0] += 128
                    nc.tensor.matmul(ps[:], Id_t[:], t, start=(r == 0), stop=False)
                for r in range(rb):
                    t = get_tile("B", pB[0], idxB_t, XB_d[:]); pB[0] += 128
                    nc.tensor.matmul(ps[:], Id_t[:], t,
                                     start=False, stop=(r == rb - 1))
                s_sb = wp.tile([128, 128], BF16, tag="s_sb", name=f"s_sb{w}")
                nc.scalar.copy(s_sb[:], ps[:])
                pst = psT.tile([128, 128], F32, tag="t", name=f"psT{w}")
                nc.tensor.matmul(pst[:], s_sb[:], Id_t[:], start=True, stop=True)
                st_sb = wp.tile([128, 128], BF16, tag="st_sb", name=f"st_sb{w}")
                nc.scalar.copy(st_sb[:], pst[:])
                pxe = psF.tile([128, 128], F32, tag="fin", name=f"psXE{w}")
                nc.tensor.matmul(pxe[:], st_sb[:], K1_t[:], start=True, stop=False)
                nc.tensor.matmul(pxe[:], auxe_t[:, w * 128:(w + 1) * 128], K2_t[:],
                                 start=False, stop=True)
                xe_sb = wp.tile([128, D], BF16, tag="xe_sb", name=f"xe_sb{w}")
                nc.scalar.activation(
                    out=xe_sb[:], in_=pxe[:],
                    func=mybir.ActivationFunctionType.Copy,
                    scale=invde_t[:, w:w + 1])
                if w * 128 < H:
                    nc.sync.dma_start(xe_lo[w * 128:(w + 1) * 128, :], xe_sb[:])
                else:
                    nc.sync.dma_start(xe_hi[w * 128 - H:(w + 1) * 128 - H, :], xe_sb[:])

            # ============ allgather (two halves, first overlaps stage-1 tail)
            nc.gpsimd.collective_compute(
                "AllGather", mybir.AluOpType.bypass,
                replica_groups=[list(range(C))],
                ins=[xe_lo.opt()], outs=[xe_all_lo.opt()])
            nc.gpsimd.collective_compute(
                "AllGather", mybir.AluOpType.bypass,
                replica_groups=[list(range(C))],
                ins=[xe_hi.opt()], outs=[xe_all_hi.opt()])
            for cc in range(C):
                nc.sync.dma_start(xe_tab[cc * EP: cc * EP + H, :],
                                  xe_all_lo[cc * H:(cc + 1) * H, :])
                nc.sync.dma_start(xe_tab[cc * EP + H:(cc + 1) * EP, :],
                                  xe_all_hi[cc * (EP - H):(cc + 1) * (EP - H), :])

            # ============ stage 2 ============
            p2 = [0]
            for w in range(NWIN_V):
                sl = slice(w * 128, (w + 1) * 128)
                r2 = R2[w]
                pre = psA.tile([128, 128], F32, tag="acc", name=f"psP{w}")
                for r in range(r2):
                    t = get_tile("2", p2[0], idx2_t, xe_tab[:]); p2[0] += 128
                    nc.tensor.matmul(pre[:], diag_t[:, w, :], t,
                                     start=(r == 0), stop=False)
                nc.tensor.matmul(pre[:], XT_t[:, sl], MX_t[:], start=False, stop=False)
                nc.tensor.matmul(pre[:], X0T_t[:, sl], MX0_t[:], start=False, stop=False)
                nc.tensor.matmul(pre[:], auxv_t[:, sl], RC2_t[:], start=False, stop=True)
                relu_sb = wp.tile([128, 128], BF16, tag="relu", name=f"relu{w}")
                nc.scalar.activation(out=relu_sb[:], in_=pre[:],
                                     func=mybir.ActivationFunctionType.Relu)
                prt = psT.tile([128, 128], F32, tag="t", name=f"psRT{w}")
                nc.tensor.matmul(prt[:], relu_sb[:], Id_t[:], start=True, stop=True)
                rt_sb = wp.tile([128, 128], BF16, tag="rt", name=f"rt{w}")
                nc.scalar.copy(rt_sb[:], prt[:])
                pso = psF.tile([128, 128], F32, tag="fin", name=f"psO{w}")
                nc.tensor.matmul(pso[:], rt_sb[:], W3w2_t[:], start=True, stop=False)
                nc.tensor.matmul(pso[:], ones1_t[:], b2row_t[:], start=False, stop=True)
                o_sb = wp.tile([128, D], F32, tag="o_sb", name=f"o_sb{w}")
                nc.scalar.copy(o_sb[:], pso[:])
                nc.sync.dma_start(out_d[sl, :], o_sb[:])

    nc.finalize()
    return nc


def run(trace=False, nq=NQ, **inputs):
    in_maps, sched, unperm = prepare(inputs)
    nc = build(in_maps[0], sched, nq=nq)
    res = run_bass_kernel_spmd(nc, in_maps, list(range(C)), trace=trace)
    out = np.empty((N, D), np.float32)
    v_core, v_pos = unperm["v_core"], unperm["v_pos"]
    for c in range(C):
        oc = res.results[c]["out"]          # [VP, D]
        mask = v_core == c
        out[mask] = oc[v_pos[mask]]
    return out, res


def kernel(**inputs):
    """Harness entry point: full inputs in, full [N, D] float32 output."""
    out, _res = run(trace=False, **inputs)
    return out.astype(np.float32)


# revision 10
# speedup vs baseline: 1.3968x; 1.3968x over previous
"""Trainium2 Bass kernel for nn_MeanDegConv (gnn_message_passing) on 8 NeuronCores.

Round-based design: incidences are laid out as (window, round, slot) grids so
segment sums become PSUM-accumulating identity/diag matmuls (no per-tile
one-hot builds on the vector engine). Gather tables are bf16 (halved DMA
bytes) and gather descriptor generation rotates across SWDGE queues.

Self-contained: imports the Bass/Tile stack from /opt/trn_rl_repo (part of the
container environment) and hardcodes all shapes/sharding for the problem.
"""
import sys
for _p in ('/opt/trn_rl_repo',):
    if _p not in sys.path:
        sys.path.insert(0, _p)

import numpy as np

import concourse.bass as bass
import concourse.mybir as mybir
import concourse.tile as tile
import concourse.bacc as bacc
from concourse.bass_utils import run_bass_kernel_spmd

N, E, NNZ, D = 50000, 10000, 1000000, 128
C = 8
EPC, VPC = E // C, N // C          # 1250 edges, 6250 vertices per core
NWIN_E = (EPC + 127) // 128        # 10
NWIN_V = (VPC + 127) // 128        # 49
EP = NWIN_E * 128                  # 1280 padded edge slots per core
VP = NWIN_V * 128                  # 6272 padded vertex slots per core
CHUNK = 4096                       # gather indices per dma_gather call
TPC = CHUNK // 128                 # tiles per chunk
NQ = 4                             # SWDGE queues to rotate desc-gen across

SPLA = 32639                       # XA covers vertices [0, 32639); zero block after
XB_BASE = N - SPLA                 # 17361; XB covers [17361, 50000); zero block after
XTAB_ROWS = SPLA + 128             # 32767 rows per split table (128 zero rows)
XE_ROWS = C * EP                   # 10240 real xe rows
XE_ZERO = XE_ROWS                  # zero block start in xe_tab
XE_TAB_ROWS = XE_ROWS + 128        # 128 zero rows

F32 = mybir.dt.float32
BF16 = mybir.dt.bfloat16
I16 = mybir.dt.int16
BFNP = mybir.dt.np(BF16)


def _pack_idx16(idx32: np.ndarray) -> np.ndarray:
    """[L] int32 -> [128, L/16] int16 in the dma_gather wrap layout."""
    L = len(idx32)
    assert L % 16 == 0
    a = idx32.astype(np.int16).reshape(L // 16, 16).T  # [16, L/16]
    return np.ascontiguousarray(np.tile(a, (8, 1)))    # [128, L/16]


def _padlen(L):
    return ((L + CHUNK - 1) // CHUNK) * CHUNK


def prepare(inputs):
    X = np.asarray(inputs["X"], np.float32)
    X0 = np.asarray(inputs["X0"], np.float32)
    v = np.asarray(inputs["vertex"]).astype(np.int64)
    e = np.asarray(inputs["edges"]).astype(np.int64)
    W1_w = np.asarray(inputs["W1_w"], np.float32); W1_b = np.asarray(inputs["W1_b"], np.float32)
    W2_w = np.asarray(inputs["W2_w"], np.float32); W2_b = np.asarray(inputs["W2_b"], np.float32)
    W3_w1 = np.asarray(inputs["W3_w1"], np.float32); W3_b1 = np.asarray(inputs["W3_b1"], np.float32)
    W3_w2 = np.asarray(inputs["W3_w2"], np.float32); W3_b2 = np.asarray(inputs["W3_b2"], np.float32)

    deg_e = np.bincount(e, minlength=E)
    deg_v = np.bincount(v, minlength=N)

    # ---- folded weight matrices (float64 for accuracy, cast at the end)
    W2a = W2_w[:D].astype(np.float64); W2b1 = W2_w[D:2*D].astype(np.float64)
    w2b_log = W2_w[2*D].astype(np.float64)
    R1 = W3_w1[:D].astype(np.float64); R2 = W3_w1[D:2*D].astype(np.float64)
    R3 = W3_w1[2*D:3*D].astype(np.float64); r4 = W3_w1[3*D].astype(np.float64)
    W2bR = W2b1 @ R1
    K1 = (W1_w.astype(np.float64) @ W2bR).astype(np.float32)
    k2 = (w2b_log @ R1).astype(np.float32)
    c1 = (W1_b.astype(np.float64) @ W2bR).astype(np.float32)
    MX = (W2a @ R1 + R2).astype(np.float32)
    MX0 = R3.astype(np.float32)
    c0 = (W2_b.astype(np.float64) @ R1 + W3_b1).astype(np.float32)

    # ---- permutations: sort by degree desc, deal round-robin to cores
    eperm = np.argsort(-deg_e, kind="stable")
    e_core = np.empty(E, np.int64); e_pos = np.empty(E, np.int64)
    e_core[eperm] = np.arange(E) % C
    e_pos[eperm] = np.arange(E) // C
    vperm = np.argsort(-deg_v, kind="stable")
    v_core = np.empty(N, np.int64); v_pos = np.empty(N, np.int64)
    v_core[vperm] = np.arange(N) % C
    v_pos[vperm] = np.arange(N) // C

    # ---- stage 1: A/B balanced split per edge
    cls = np.where(v < XB_BASE, 0, np.where(v >= SPLA, 2, 1))
    nAf = np.bincount(e[cls == 0], minlength=E)
    nBf = np.bincount(e[cls == 2], minlength=E)
    cntA = np.clip((deg_e + 1) // 2, nAf, deg_e - nBf)

    cA = np.zeros((C, EP), np.int64); cB = np.zeros((C, EP), np.int64)
    cA[e_core, e_pos] = cntA
    cB[e_core, e_pos] = deg_e - cntA
    RA = cA.reshape(C, NWIN_E, 128).max(axis=(0, 2))
    RB = cB.reshape(C, NWIN_E, 128).max(axis=(0, 2))
    LA = int(RA.sum()) * 128
    LB = int(RB.sum()) * 128

    # order incidences by (edge, class): forced-A, middles, forced-B
    oinc = np.argsort(e * 4 + cls, kind="stable")
    e_s = e[oinc]; v_s = v[oinc]
    starts = np.searchsorted(e_s, np.arange(E))
    rank = np.arange(NNZ) - starts[e_s]
    isA = rank < cntA[e_s]
    ecore1 = e_core[e_s]; epos1 = e_pos[e_s]
    w1 = epos1 // 128; s1 = epos1 % 128
    offA = np.zeros(NWIN_E, np.int64); offA[1:] = np.cumsum(RA)[:-1]
    offB = np.zeros(NWIN_E, np.int64); offB[1:] = np.cumsum(RB)[:-1]
    posA = (offA[w1] + rank) * 128 + s1
    posB = (offB[w1] + (rank - cntA[e_s])) * 128 + s1

    # ---- stage 2 rounds
    cV = np.zeros((C, VP), np.int64)
    cV[v_core, v_pos] = deg_v
    R2r = cV.reshape(C, NWIN_V, 128).max(axis=(0, 2))
    L2 = int(R2r.sum()) * 128

    rowid_of_e = e_core * EP + e_pos
    o2 = np.argsort(v, kind="stable")
    v_s2 = v[o2]; e_s2 = e[o2]
    starts2 = np.searchsorted(v_s2, np.arange(N))
    rank2 = np.arange(NNZ) - starts2[v_s2]
    vcore2 = v_core[v_s2]; vpos2 = v_pos[v_s2]
    w2 = vpos2 // 128; s2w = vpos2 % 128
    off2 = np.zeros(NWIN_V, np.int64); off2[1:] = np.cumsum(R2r)[:-1]
    pos2 = (off2[w2] + rank2) * 128 + s2w
    val2 = rowid_of_e[e_s2]

    sched = {"RA": [int(x) for x in RA], "RB": [int(x) for x in RB],
             "R2": [int(x) for x in R2r],
             "LA": _padlen(LA), "LB": _padlen(LB), "L2": _padlen(L2)}

    # ---- shared consts
    Xb = X.astype(BFNP)
    XA = np.zeros((XTAB_ROWS, D), BFNP); XA[:SPLA] = Xb[:SPLA]
    XBt = np.zeros((XTAB_ROWS, D), BFNP); XBt[:N - XB_BASE] = Xb[XB_BASE:]
    deg_ef = deg_e.astype(np.float32); log_deg_e = np.log(deg_ef)
    deg_vf = deg_v.astype(np.float32); log_deg_v = np.log(deg_vf)

    consts = {
        "XA": XA, "XB": XBt,
        "iota": np.tile(np.arange(128, dtype=np.float32), (128, 1)).astype(BFNP),
        "iotacol": np.arange(128, dtype=np.float32).reshape(128, 1),
        "K1": K1.astype(BFNP),
        "K2": np.ascontiguousarray(np.stack([k2, c1])),
        "MX": MX, "MX0": MX0,
        "RC2": np.ascontiguousarray(np.stack([r4.astype(np.float32), c0])),
        "W3w2": W3_w2.astype(BFNP),
        "b2row": W3_b2.reshape(1, D),
        "ones1": np.ones((1, 128), np.float32),
    }

    edge_at = np.full((C, EP), -1, np.int64)
    edge_at[e_core, e_pos] = np.arange(E)
    vert_at = np.full((C, VP), -1, np.int64)
    vert_at[v_core, v_pos] = np.arange(N)

    in_maps = []
    for c in range(C):
        mA = (ecore1 == c) & isA
        mB = (ecore1 == c) & (~isA)
        sA = SPLA + (np.arange(sched["LA"], dtype=np.int32) % 128)
        sA[posA[mA]] = v_s[mA]
        sB = SPLA + (np.arange(sched["LB"], dtype=np.int32) % 128)
        sB[posB[mB]] = v_s[mB] - XB_BASE

        m2 = vcore2 == c
        s2 = XE_ZERO + (np.arange(sched["L2"], dtype=np.int32) % 128)
        s2[pos2[m2]] = val2[m2]

        de = np.ones(EP, np.float32); le = np.zeros(EP, np.float32)
        msk = edge_at[c] >= 0
        de[msk] = deg_ef[edge_at[c][msk]]
        le[msk] = log_deg_e[edge_at[c][msk]]
        auxe = np.ascontiguousarray(np.stack([de * le, de]))
        invde_col = np.ascontiguousarray((1.0 / de).reshape(NWIN_E, 128).T)

        dv = np.ones(VP, np.float32); lv = np.zeros(VP, np.float32)
        vm = vert_at[c] >= 0
        dv[vm] = deg_vf[vert_at[c][vm]]
        lv[vm] = log_deg_v[vert_at[c][vm]]
        auxv = np.ascontiguousarray(np.stack([lv, np.ones(VP, np.float32)]))
        invdv_col = np.ascontiguousarray((1.0 / dv).reshape(NWIN_V, 128).T)

        Xp = np.zeros((VP, D), np.float32); X0p = np.zeros((VP, D), np.float32)
        Xp[vm] = X[vert_at[c][vm]]
        X0p[vm] = X0[vert_at[c][vm]]

        m = dict(consts)
        m.update({
            "idxA": _pack_idx16(sA), "idxB": _pack_idx16(sB),
            "idx2": _pack_idx16(s2),
            "auxe": auxe, "invde_col": invde_col,
            "auxv": auxv, "invdv_col": invdv_col,
            "XT": np.ascontiguousarray(Xp.T), "X0T": np.ascontiguousarray(X0p.T),
        })
        in_maps.append(m)
    unperm = {"v_core": v_core, "v_pos": v_pos}
    return in_maps, sched, unperm


def build(in_map0, sched, nq=NQ):
    RA, RB, R2 = sched["RA"], sched["RB"], sched["R2"]
    nc = bacc.Bacc(None, num_swdge_queues=nq, dynamic_dma_scratch_size=32768)

    def param(name, dt=F32):
        arr = in_map0[name]
        return nc.declare_dram_parameter(name, list(arr.shape), dt, isOutput=False)

    XA_d = param("XA", BF16); XB_d = param("XB", BF16)
    iota_d = param("iota", BF16); iotacol_d = param("iotacol")
    K1_d = param("K1", BF16); K2_d = param("K2")
    MX_d = param("MX"); MX0_d = param("MX0"); RC2_d = param("RC2")
    W3w2_d = param("W3w2", BF16); b2row_d = param("b2row"); ones1_d = param("ones1")
    idxA_d = param("idxA", I16); idxB_d = param("idxB", I16); idx2_d = param("idx2", I16)
    auxe_d = param("auxe"); invde_d = param("invde_col")
    auxv_d = param("auxv"); invdv_d = param("invdv_col")
    XT_d = param("XT"); X0T_d = param("X0T")
    out_d = nc.declare_dram_parameter("out", [VP, D], F32, isOutput=True)

    # Rotate desc-gen across SWDGE queues 1..3: their Q7 core-pairs generate
    # descriptors off the Pool engine's critical path (queue 0 blocks ~32us).
    qrot = [1, 2, 3] if nq == 4 else list(range(nq))
    qctr = [0]

    def next_q():
        q = qrot[qctr[0] % len(qrot)]
        qctr[0] += 1
        return q

    with tile.TileContext(nc) as tc:
        with (
            tc.tile_pool(name="const", bufs=1) as cp,
            tc.tile_pool(name="stream", bufs=1) as sp,
            tc.tile_pool(name="g", bufs=6) as gp,
            tc.tile_pool(name="work", bufs=3) as wp,
            tc.tile_pool(name="psA", bufs=2, space="PSUM") as psA,
            tc.tile_pool(name="psT", bufs=2, space="PSUM") as psT,
            tc.tile_pool(name="psF", bufs=2, space="PSUM") as psF,
            tc.tile_pool(name="dram", bufs=1, space="DRAM") as dp,
        ):
            def load(pool, dram_ap, name, dt=F32, eng=None):
                t = pool.tile(list(dram_ap.shape), dt, name=name, tag=name)
                (eng or nc.sync).dma_start(t[:], dram_ap[:])
                return t

            idxA_t = load(sp, idxA_d, "idxA", I16)
            idxB_t = load(sp, idxB_d, "idxB", I16)
            idx2_t = load(sp, idx2_d, "idx2", I16)
            iota_t = load(cp, iota_d, "iota", BF16)
            iotacol_t = load(cp, iotacol_d, "iotacol")
            K1_t = load(cp, K1_d, "K1", BF16); K2_t = load(cp, K2_d, "K2")
            MX_t = load(cp, MX_d, "MX"); MX0_t = load(cp, MX0_d, "MX0")
            RC2_t = load(cp, RC2_d, "RC2")
            W3w2_t = load(cp, W3w2_d, "W3w2", BF16)
            b2row_t = load(cp, b2row_d, "b2row"); ones1_t = load(cp, ones1_d, "ones1")
            auxe_t = load(cp, auxe_d, "auxe"); invde_t = load(cp, invde_d, "invde")
            auxv_t = load(cp, auxv_d, "auxv"); invdv_t = load(cp, invdv_d, "invdv")
            XT_t = load(cp, XT_d, "XT", eng=nc.scalar)
            X0T_t = load(cp, X0T_d, "X0T", eng=nc.scalar)

            # identity (bf16): Id[s, j] = (iota[s, j] == s)
            Id_t = cp.tile([128, 128], BF16, name="Id", tag="Id")
            nc.vector.tensor_scalar(
                out=Id_t[:], in0=iota_t[:], scalar1=iotacol_t[:, 0:1],
                scalar2=None, op0=mybir.AluOpType.is_equal)
            # per-vertex-window diag(1/deg_v) bf16
            diag_t = cp.tile([128, NWIN_V, 128], BF16, name="diag", tag="diag")
            for w in range(NWIN_V):
                nc.vector.tensor_scalar(
                    out=diag_t[:, w, :], in0=iota_t[:], scalar1=iotacol_t[:, 0:1],
                    scalar2=invdv_t[:, w:w + 1], op0=mybir.AluOpType.is_equal,
                    op1=mybir.AluOpType.mult)

            H = (NWIN_E // 2) * 128
            xe_lo = dp.tile([H, D], BF16)
            xe_hi = dp.tile([EP - H, D], BF16)
            xe_all_lo = dp.tile([C * (NWIN_E // 2) * 128, D], BF16, addr_space="Shared")
            xe_all_hi = dp.tile([C * (EP - (NWIN_E // 2) * 128), D], BF16, addr_space="Shared")
            xe_tab = dp.tile([XE_TAB_ROWS, D], BF16)
            zrow = wp.tile([128, D], BF16, tag="zrow", name="zrow")
            nc.vector.memset(zrow[:], 0.0)
            nc.sync.dma_start(xe_tab[XE_ZERO:XE_ZERO + 128, :], zrow[:])

            # ---- lazy chunked gathers: one shared SBUF ring, issue on demand
            chunks = {}

            def get_tile(stream, pos, idx_t, in_ap):
                lst = chunks.setdefault(stream, [])
                ci = pos // CHUNK
                while len(lst) <= ci:
                    k = len(lst)
                    g = gp.tile([128, TPC, D], BF16, tag="g", name=f"g{stream}{k}")
                    nc.gpsimd.dma_gather(
                        out_ap=g[:], in_ap=in_ap,
                        idxs_ap=idx_t[:, k * (CHUNK // 16):(k + 1) * (CHUNK // 16)],
                        num_idxs=CHUNK, num_idxs_reg=CHUNK,
                        single_packet=False, elem_size=D, queue_num=next_q())
                    lst.append(g)
                return lst[ci][:, (pos % CHUNK) // 128, :]

            # ============ stage 1 ============
            pA = [0]; pB = [0]
            for w in range(NWIN_E):
                ra, rb = RA[w], RB[w]
                ps = psA.tile([128, 128], F32, tag="acc", name=f"psS{w}")
                for r in range(ra):
                    t = get_tile("A", pA[0], idxA_t, XA_d[:]); pA[0] += 128
                    nc.tensor.matmul(ps[:], Id_t[:], t, start=(r == 0), stop=False)
                for r in range(rb):
                    t = get_tile("B", pB[0], idxB_t, XB_d[:]); pB[0] += 128
                    nc.tensor.matmul(ps[:], Id_t[:], t,
                                     start=False, stop=(r == rb - 1))
                s_sb = wp.tile([128, 128], BF16, tag="s_sb", name=f"s_sb{w}")
                nc.scalar.copy(s_sb[:], ps[:])
                pst = psT.tile([128, 128], F32, tag="t", name=f"psT{w}")
                nc.tensor.matmul(pst[:], s_sb[:], Id_t[:], start=True, stop=True)
                st_sb = wp.tile([128, 128], BF16, tag="st_sb", name=f"st_sb{w}")
                nc.scalar.copy(st_sb[:], pst[:])
                pxe = psF.tile([128, 128], F32, tag="fin", name=f"psXE{w}")
                nc.tensor.matmul(pxe[:], st_sb[:], K1_t[:], start=True, stop=False)
                nc.tensor.matmul(pxe[:], auxe_t[:, w * 128:(w + 1) * 128], K2_t[:],
                                 start=False, stop=True)
                xe_sb = wp.tile([128, D], BF16, tag="xe_sb", name=f"xe_sb{w}")
                nc.scalar.activation(
                    out=xe_sb[:], in_=pxe[:],
                    func=mybir.ActivationFunctionType.Copy,
                    scale=invde_t[:, w:w + 1])
                if w * 128 < H:
                    nc.sync.dma_start(xe_lo[w * 128:(w + 1) * 128, :], xe_sb[:])
                else:
                    nc.sync.dma_start(xe_hi[w * 128 - H:(w + 1) * 128 - H, :], xe_sb[:])

            # ============ allgather (two halves, first overlaps stage-1 tail)
            nc.gpsimd.collective_compute(
                "AllGather", mybir.AluOpType.bypass,
                replica_groups=[list(range(C))],
                ins=[xe_lo.opt()], outs=[xe_all_lo.opt()])
            nc.gpsimd.collective_compute(
                "AllGather", mybir.AluOpType.bypass,
                replica_groups=[list(range(C))],
                ins=[xe_hi.opt()], outs=[xe_all_hi.opt()])
            for cc in range(C):
                nc.sync.dma_start(xe_tab[cc * EP: cc * EP + H, :],
                                  xe_all_lo[cc * H:(cc + 1) * H, :])
                nc.sync.dma_start(xe_tab[cc * EP + H:(cc + 1) * EP, :],
                                  xe_all_hi[cc * (EP - H):(cc + 1) * (EP - H), :])

            # ============ stage 2 ============
            p2 = [0]
            for w in range(NWIN_V):
                sl = slice(w * 128, (w + 1) * 128)
                r2 = R2[w]
                pre = psA.tile([128, 128], F32, tag="acc", name=f"psP{w}")
                for r in range(r2):
                    t = get_tile("2", p2[0], idx2_t, xe_tab[:]); p2[0] += 128
                    nc.tensor.matmul(pre[:], diag_t[:, w, :], t,
                                     start=(r == 0), stop=False)
                nc.tensor.matmul(pre[:], XT_t[:, sl], MX_t[:], start=False, stop=False)
                nc.tensor.matmul(pre[:], X0T_t[:, sl], MX0_t[:], start=False, stop=False)
                nc.tensor.matmul(pre[:], auxv_t[:, sl], RC2_t[:], start=False, stop=True)
                relu_sb = wp.tile([128, 128], BF16, tag="relu", name=f"relu{w}")
                nc.scalar.activation(out=relu_sb[:], in_=pre[:],
                                     func=mybir.ActivationFunctionType.Relu)
                prt = psT.tile([128, 128], F32, tag="t", name=f"psRT{w}")
                nc.tensor.matmul(prt[:], relu_sb[:], Id_t[:], start=True, stop=True)
                rt_sb = wp.tile([128, 128], BF16, tag="rt", name=f"rt{w}")
                nc.scalar.copy(rt_sb[:], prt[:])
                pso = psF.tile([128, 128], F32, tag="fin", name=f"psO{w}")
                nc.tensor.matmul(pso[:], rt_sb[:], W3w2_t[:], start=True, stop=False)
                nc.tensor.matmul(pso[:], ones1_t[:], b2row_t[:], start=False, stop=True)
                o_sb = wp.tile([128, D], F32, tag="o_sb", name=f"o_sb{w}")
                nc.scalar.copy(o_sb[:], pso[:])
                nc.sync.dma_start(out_d[sl, :], o_sb[:])

    nc.finalize()
    return nc


def run(trace=False, nq=NQ, **inputs):
    in_maps, sched, unperm = prepare(inputs)
    nc = build(in_maps[0], sched, nq=nq)
    res = run_bass_kernel_spmd(nc, in_maps, list(range(C)), trace=trace)
    out = np.empty((N, D), np.float32)
    v_core, v_pos = unperm["v_core"], unperm["v_pos"]
    for c in range(C):
        oc = res.results[c]["out"]          # [VP, D]
        mask = v_core == c
        out[mask] = oc[v_pos[mask]]
    return out, res


def kernel(**inputs):
    """Harness entry point: full inputs in, full [N, D] float32 output."""
    out, _res = run(trace=False, **inputs)
    return out.astype(np.float32)


# revision 11
# speedup vs baseline: 1.4116x; 1.0106x over previous
"""Trainium2 Bass kernel for nn_MeanDegConv (gnn_message_passing) on 8 NeuronCores.

Round-based design: incidences are laid out as (window, round, slot) grids so
segment sums become PSUM-accumulating identity/diag matmuls (no per-tile
one-hot builds on the vector engine). Gather tables are bf16 (halved DMA
bytes) and gather descriptor generation rotates across SWDGE queues.

Self-contained: imports the Bass/Tile stack from /opt/trn_rl_repo (part of the
container environment) and hardcodes all shapes/sharding for the problem.
"""
import sys
for _p in ('/opt/trn_rl_repo',):
    if _p not in sys.path:
        sys.path.insert(0, _p)

import numpy as np

import concourse.bass as bass
import concourse.mybir as mybir
import concourse.tile as tile
import concourse.bacc as bacc
from concourse.bass_utils import run_bass_kernel_spmd

N, E, NNZ, D = 50000, 10000, 1000000, 128
C = 8
EPC, VPC = E // C, N // C          # 1250 edges, 6250 vertices per core
NWIN_E = (EPC + 127) // 128        # 10
NWIN_V = (VPC + 127) // 128        # 49
EP = NWIN_E * 128                  # 1280 padded edge slots per core
VP = NWIN_V * 128                  # 6272 padded vertex slots per core
CHUNK = 4096                       # gather indices per dma_gather call
TPC = CHUNK // 128                 # tiles per chunk
NQ = 4                             # SWDGE queues to rotate desc-gen across

SPLA = 32639                       # XA covers vertices [0, 32639); zero block after
XB_BASE = N - SPLA                 # 17361; XB covers [17361, 50000); zero block after
XTAB_ROWS = SPLA + 128             # 32767 rows per split table (128 zero rows)
XE_ROWS = C * EP                   # 10240 real xe rows
XE_ZERO = XE_ROWS                  # zero block start in xe_tab
XE_TAB_ROWS = XE_ROWS + 128        # 128 zero rows

F32 = mybir.dt.float32
BF16 = mybir.dt.bfloat16
I16 = mybir.dt.int16
BFNP = mybir.dt.np(BF16)


def _pack_idx16(idx32: np.ndarray) -> np.ndarray:
    """[L] int32 -> [128, L/16] int16 in the dma_gather wrap layout."""
    L = len(idx32)
    assert L % 16 == 0
    a = idx32.astype(np.int16).reshape(L // 16, 16).T  # [16, L/16]
    return np.ascontiguousarray(np.tile(a, (8, 1)))    # [128, L/16]


def _padlen(L):
    return ((L + CHUNK - 1) // CHUNK) * CHUNK


def prepare(inputs):
    X = np.asarray(inputs["X"], np.float32)
    X0 = np.asarray(inputs["X0"], np.float32)
    v = np.asarray(inputs["vertex"]).astype(np.int64)
    e = np.asarray(inputs["edges"]).astype(np.int64)
    W1_w = np.asarray(inputs["W1_w"], np.float32); W1_b = np.asarray(inputs["W1_b"], np.float32)
    W2_w = np.asarray(inputs["W2_w"], np.float32); W2_b = np.asarray(inputs["W2_b"], np.float32)
    W3_w1 = np.asarray(inputs["W3_w1"], np.float32); W3_b1 = np.asarray(inputs["W3_b1"], np.float32)
    W3_w2 = np.asarray(inputs["W3_w2"], np.float32); W3_b2 = np.asarray(inputs["W3_b2"], np.float32)

    deg_e = np.bincount(e, minlength=E)
    deg_v = np.bincount(v, minlength=N)

    # ---- folded weight matrices (float64 for accuracy, cast at the end)
    W2a = W2_w[:D].astype(np.float64); W2b1 = W2_w[D:2*D].astype(np.float64)
    w2b_log = W2_w[2*D].astype(np.float64)
    R1 = W3_w1[:D].astype(np.float64); R2 = W3_w1[D:2*D].astype(np.float64)
    R3 = W3_w1[2*D:3*D].astype(np.float64); r4 = W3_w1[3*D].astype(np.float64)
    W2bR = W2b1 @ R1
    K1 = (W1_w.astype(np.float64) @ W2bR).astype(np.float32)
    k2 = (w2b_log @ R1).astype(np.float32)
    c1 = (W1_b.astype(np.float64) @ W2bR).astype(np.float32)
    MX = (W2a @ R1 + R2).astype(np.float32)
    MX0 = R3.astype(np.float32)
    c0 = (W2_b.astype(np.float64) @ R1 + W3_b1).astype(np.float32)

    # ---- permutations: sort by degree desc, deal round-robin to cores
    eperm = np.argsort(-deg_e, kind="stable")
    e_core = np.empty(E, np.int64); e_pos = np.empty(E, np.int64)
    e_core[eperm] = np.arange(E) % C
    e_pos[eperm] = np.arange(E) // C
    vperm = np.argsort(-deg_v, kind="stable")
    v_core = np.empty(N, np.int64); v_pos = np.empty(N, np.int64)
    v_core[vperm] = np.arange(N) % C
    v_pos[vperm] = np.arange(N) // C

    # ---- stage 1: A/B balanced split per edge
    cls = np.where(v < XB_BASE, 0, np.where(v >= SPLA, 2, 1))
    nAf = np.bincount(e[cls == 0], minlength=E)
    nBf = np.bincount(e[cls == 2], minlength=E)
    cntA = np.clip((deg_e + 1) // 2, nAf, deg_e - nBf)

    cA = np.zeros((C, EP), np.int64); cB = np.zeros((C, EP), np.int64)
    cA[e_core, e_pos] = cntA
    cB[e_core, e_pos] = deg_e - cntA
    RA = cA.reshape(C, NWIN_E, 128).max(axis=(0, 2))
    RB = cB.reshape(C, NWIN_E, 128).max(axis=(0, 2))
    LA = int(RA.sum()) * 128
    LB = int(RB.sum()) * 128

    # order incidences by (edge, class): forced-A, middles, forced-B
    oinc = np.argsort(e * 4 + cls, kind="stable")
    e_s = e[oinc]; v_s = v[oinc]
    starts = np.searchsorted(e_s, np.arange(E))
    rank = np.arange(NNZ) - starts[e_s]
    isA = rank < cntA[e_s]
    ecore1 = e_core[e_s]; epos1 = e_pos[e_s]
    w1 = epos1 // 128; s1 = epos1 % 128
    offA = np.zeros(NWIN_E, np.int64); offA[1:] = np.cumsum(RA)[:-1]
    offB = np.zeros(NWIN_E, np.int64); offB[1:] = np.cumsum(RB)[:-1]
    posA = (offA[w1] + rank) * 128 + s1
    posB = (offB[w1] + (rank - cntA[e_s])) * 128 + s1

    # ---- stage 2 rounds
    cV = np.zeros((C, VP), np.int64)
    cV[v_core, v_pos] = deg_v
    R2r = cV.reshape(C, NWIN_V, 128).max(axis=(0, 2))
    L2 = int(R2r.sum()) * 128

    rowid_of_e = e_core * EP + e_pos
    o2 = np.argsort(v, kind="stable")
    v_s2 = v[o2]; e_s2 = e[o2]
    starts2 = np.searchsorted(v_s2, np.arange(N))
    rank2 = np.arange(NNZ) - starts2[v_s2]
    vcore2 = v_core[v_s2]; vpos2 = v_pos[v_s2]
    w2 = vpos2 // 128; s2w = vpos2 % 128
    off2 = np.zeros(NWIN_V, np.int64); off2[1:] = np.cumsum(R2r)[:-1]
    pos2 = (off2[w2] + rank2) * 128 + s2w
    val2 = rowid_of_e[e_s2]

    sched = {"RA": [int(x) for x in RA], "RB": [int(x) for x in RB],
             "R2": [int(x) for x in R2r],
             "LA": _padlen(LA), "LB": _padlen(LB), "L2": _padlen(L2)}

    # ---- shared consts
    Xb = X.astype(BFNP)
    XA = np.zeros((XTAB_ROWS, D), BFNP); XA[:SPLA] = Xb[:SPLA]
    XBt = np.zeros((XTAB_ROWS, D), BFNP); XBt[:N - XB_BASE] = Xb[XB_BASE:]
    deg_ef = deg_e.astype(np.float32); log_deg_e = np.log(deg_ef)
    deg_vf = deg_v.astype(np.float32); log_deg_v = np.log(deg_vf)

    consts = {
        "XA": XA, "XB": XBt,
        "iota": np.tile(np.arange(128, dtype=np.float32), (128, 1)).astype(BFNP),
        "iotacol": np.arange(128, dtype=np.float32).reshape(128, 1),
        "K1": K1.astype(BFNP),
        "K2": np.ascontiguousarray(np.stack([k2, c1])),
        "MX": MX, "MX0": MX0,
        "RC2": np.ascontiguousarray(np.stack([r4.astype(np.float32), c0])),
        "W3w2": W3_w2.astype(BFNP),
        "b2row": W3_b2.reshape(1, D),
        "ones1": np.ones((1, 128), np.float32),
    }

    edge_at = np.full((C, EP), -1, np.int64)
    edge_at[e_core, e_pos] = np.arange(E)
    vert_at = np.full((C, VP), -1, np.int64)
    vert_at[v_core, v_pos] = np.arange(N)

    in_maps = []
    for c in range(C):
        mA = (ecore1 == c) & isA
        mB = (ecore1 == c) & (~isA)
        sA = SPLA + (np.arange(sched["LA"], dtype=np.int32) % 128)
        sA[posA[mA]] = v_s[mA]
        sB = SPLA + (np.arange(sched["LB"], dtype=np.int32) % 128)
        sB[posB[mB]] = v_s[mB] - XB_BASE

        m2 = vcore2 == c
        s2 = XE_ZERO + (np.arange(sched["L2"], dtype=np.int32) % 128)
        s2[pos2[m2]] = val2[m2]

        de = np.ones(EP, np.float32); le = np.zeros(EP, np.float32)
        msk = edge_at[c] >= 0
        de[msk] = deg_ef[edge_at[c][msk]]
        le[msk] = log_deg_e[edge_at[c][msk]]
        auxe = np.ascontiguousarray(np.stack([de * le, de]))
        invde_col = np.ascontiguousarray((1.0 / de).reshape(NWIN_E, 128).T)

        dv = np.ones(VP, np.float32); lv = np.zeros(VP, np.float32)
        vm = vert_at[c] >= 0
        dv[vm] = deg_vf[vert_at[c][vm]]
        lv[vm] = log_deg_v[vert_at[c][vm]]
        auxv = np.ascontiguousarray(np.stack([lv, np.ones(VP, np.float32)]))
        invdv_col = np.ascontiguousarray((1.0 / dv).reshape(NWIN_V, 128).T)

        Xp = np.zeros((VP, D), np.float32); X0p = np.zeros((VP, D), np.float32)
        Xp[vm] = X[vert_at[c][vm]]
        X0p[vm] = X0[vert_at[c][vm]]

        m = dict(consts)
        m.update({
            "idxA": _pack_idx16(sA), "idxB": _pack_idx16(sB),
            "idx2": _pack_idx16(s2),
            "auxe": auxe, "invde_col": invde_col,
            "auxv": auxv, "invdv_col": invdv_col,
            "XT": np.ascontiguousarray(Xp.T), "X0T": np.ascontiguousarray(X0p.T),
        })
        in_maps.append(m)
    unperm = {"v_core": v_core, "v_pos": v_pos}
    return in_maps, sched, unperm


def build(in_map0, sched, nq=NQ):
    RA, RB, R2 = sched["RA"], sched["RB"], sched["R2"]
    nc = bacc.Bacc(None, num_swdge_queues=nq, dynamic_dma_scratch_size=32768)

    def param(name, dt=F32):
        arr = in_map0[name]
        return nc.declare_dram_parameter(name, list(arr.shape), dt, isOutput=False)

    XA_d = param("XA", BF16); XB_d = param("XB", BF16)
    iota_d = param("iota", BF16); iotacol_d = param("iotacol")
    K1_d = param("K1", BF16); K2_d = param("K2")
    MX_d = param("MX"); MX0_d = param("MX0"); RC2_d = param("RC2")
    W3w2_d = param("W3w2", BF16); b2row_d = param("b2row"); ones1_d = param("ones1")
    idxA_d = param("idxA", I16); idxB_d = param("idxB", I16); idx2_d = param("idx2", I16)
    auxe_d = param("auxe"); invde_d = param("invde_col")
    auxv_d = param("auxv"); invdv_d = param("invdv_col")
    XT_d = param("XT"); X0T_d = param("X0T")
    out_d = nc.declare_dram_parameter("out", [VP, D], F32, isOutput=True)

    # Rotate desc-gen across SWDGE queues 1..3: their Q7 core-pairs generate
    # descriptors off the Pool engine's critical path (queue 0 blocks ~32us).
    qrot = [1, 2, 3, 0] if nq == 4 else list(range(nq))
    qctr = [0]

    def next_q():
        q = qrot[qctr[0] % len(qrot)]
        qctr[0] += 1
        return q

    with tile.TileContext(nc) as tc:
        with (
            tc.tile_pool(name="const", bufs=1) as cp,
            tc.tile_pool(name="stream", bufs=1) as sp,
            tc.tile_pool(name="g", bufs=6) as gp,
            tc.tile_pool(name="work", bufs=3) as wp,
            tc.tile_pool(name="psA", bufs=2, space="PSUM") as psA,
            tc.tile_pool(name="psT", bufs=2, space="PSUM") as psT,
            tc.tile_pool(name="psF", bufs=2, space="PSUM") as psF,
            tc.tile_pool(name="dram", bufs=1, space="DRAM") as dp,
        ):
            def load(pool, dram_ap, name, dt=F32, eng=None):
                t = pool.tile(list(dram_ap.shape), dt, name=name, tag=name)
                (eng or nc.sync).dma_start(t[:], dram_ap[:])
                return t

            idxA_t = load(sp, idxA_d, "idxA", I16)
            idxB_t = load(sp, idxB_d, "idxB", I16)
            idx2_t = load(sp, idx2_d, "idx2", I16)
            iota_t = load(cp, iota_d, "iota", BF16)
            iotacol_t = load(cp, iotacol_d, "iotacol")
            K1_t = load(cp, K1_d, "K1", BF16); K2_t = load(cp, K2_d, "K2")
            MX_t = load(cp, MX_d, "MX"); MX0_t = load(cp, MX0_d, "MX0")
            RC2_t = load(cp, RC2_d, "RC2")
            W3w2_t = load(cp, W3w2_d, "W3w2", BF16)
            b2row_t = load(cp, b2row_d, "b2row"); ones1_t = load(cp, ones1_d, "ones1")
            auxe_t = load(cp, auxe_d, "auxe"); invde_t = load(cp, invde_d, "invde")
            auxv_t = load(cp, auxv_d, "auxv"); invdv_t = load(cp, invdv_d, "invdv")
            XT_t = load(cp, XT_d, "XT", eng=nc.scalar)
            X0T_t = load(cp, X0T_d, "X0T", eng=nc.scalar)

            # identity (bf16): Id[s, j] = (iota[s, j] == s)
            Id_t = cp.tile([128, 128], BF16, name="Id", tag="Id")
            nc.vector.tensor_scalar(
                out=Id_t[:], in0=iota_t[:], scalar1=iotacol_t[:, 0:1],
                scalar2=None, op0=mybir.AluOpType.is_equal)
            # per-vertex-window diag(1/deg_v) bf16
            diag_t = cp.tile([128, NWIN_V, 128], BF16, name="diag", tag="diag")
            for w in range(NWIN_V):
                nc.vector.tensor_scalar(
                    out=diag_t[:, w, :], in0=iota_t[:], scalar1=iotacol_t[:, 0:1],
                    scalar2=invdv_t[:, w:w + 1], op0=mybir.AluOpType.is_equal,
                    op1=mybir.AluOpType.mult)

            H = (NWIN_E // 2) * 128
            xe_lo = dp.tile([H, D], BF16)
            xe_hi = dp.tile([EP - H, D], BF16)
            xe_all_lo = dp.tile([C * (NWIN_E // 2) * 128, D], BF16, addr_space="Shared")
            xe_all_hi = dp.tile([C * (EP - (NWIN_E // 2) * 128), D], BF16, addr_space="Shared")
            xe_tab = dp.tile([XE_TAB_ROWS, D], BF16)
            zrow = wp.tile([128, D], BF16, tag="zrow", name="zrow")
            nc.vector.memset(zrow[:], 0.0)
            nc.sync.dma_start(xe_tab[XE_ZERO:XE_ZERO + 128, :], zrow[:])

            # ---- lazy chunked gathers: one shared SBUF ring, issue on demand
            chunks = {}

            def get_tile(stream, pos, idx_t, in_ap):
                lst = chunks.setdefault(stream, [])
                ci = pos // CHUNK
                while len(lst) <= ci:
                    k = len(lst)
                    g = gp.tile([128, TPC, D], BF16, tag="g", name=f"g{stream}{k}")
                    nc.gpsimd.dma_gather(
                        out_ap=g[:], in_ap=in_ap,
                        idxs_ap=idx_t[:, k * (CHUNK // 16):(k + 1) * (CHUNK // 16)],
                        num_idxs=CHUNK, num_idxs_reg=CHUNK,
                        single_packet=False, elem_size=D, queue_num=next_q())
                    lst.append(g)
                return lst[ci][:, (pos % CHUNK) // 128, :]

            # ============ stage 1 ============
            pA = [0]; pB = [0]
            for w in range(NWIN_E):
                ra, rb = RA[w], RB[w]
                ps = psA.tile([128, 128], F32, tag="acc", name=f"psS{w}")
                for r in range(ra):
                    t = get_tile("A", pA[0], idxA_t, XA_d[:]); pA[0] += 128
                    nc.tensor.matmul(ps[:], Id_t[:], t, start=(r == 0), stop=False)
                for r in range(rb):
                    t = get_tile("B", pB[0], idxB_t, XB_d[:]); pB[0] += 128
                    nc.tensor.matmul(ps[:], Id_t[:], t,
                                     start=False, stop=(r == rb - 1))
                s_sb = wp.tile([128, 128], BF16, tag="s_sb", name=f"s_sb{w}")
                nc.scalar.copy(s_sb[:], ps[:])
                pst = psT.tile([128, 128], F32, tag="t", name=f"psT{w}")
                nc.tensor.matmul(pst[:], s_sb[:], Id_t[:], start=True, stop=True)
                st_sb = wp.tile([128, 128], BF16, tag="st_sb", name=f"st_sb{w}")
                nc.scalar.copy(st_sb[:], pst[:])
                pxe = psF.tile([128, 128], F32, tag="fin", name=f"psXE{w}")
                nc.tensor.matmul(pxe[:], st_sb[:], K1_t[:], start=True, stop=False)
                nc.tensor.matmul(pxe[:], auxe_t[:, w * 128:(w + 1) * 128], K2_t[:],
                                 start=False, stop=True)
                xe_sb = wp.tile([128, D], BF16, tag="xe_sb", name=f"xe_sb{w}")
                nc.scalar.activation(
                    out=xe_sb[:], in_=pxe[:],
                    func=mybir.ActivationFunctionType.Copy,
                    scale=invde_t[:, w:w + 1])
                if w * 128 < H:
                    nc.sync.dma_start(xe_lo[w * 128:(w + 1) * 128, :], xe_sb[:])
                else:
                    nc.sync.dma_start(xe_hi[w * 128 - H:(w + 1) * 128 - H, :], xe_sb[:])

            # ============ allgather (two halves, first overlaps stage-1 tail)
            nc.gpsimd.collective_compute(
                "AllGather", mybir.AluOpType.bypass,
                replica_groups=[list(range(C))],
                ins=[xe_lo.opt()], outs=[xe_all_lo.opt()])
            nc.gpsimd.collective_compute(
                "AllGather", mybir.AluOpType.bypass,
                replica_groups=[list(range(C))],
                ins=[xe_hi.opt()], outs=[xe_all_hi.opt()])
            for cc in range(C):
                nc.sync.dma_start(xe_tab[cc * EP: cc * EP + H, :],
                                  xe_all_lo[cc * H:(cc + 1) * H, :])
                nc.sync.dma_start(xe_tab[cc * EP + H:(cc + 1) * EP, :],
                                  xe_all_hi[cc * (EP - H):(cc + 1) * (EP - H), :])

            # ============ stage 2 ============
            p2 = [0]
            for w in range(NWIN_V):
                sl = slice(w * 128, (w + 1) * 128)
                r2 = R2[w]
                pre = psA.tile([128, 128], F32, tag="acc", name=f"psP{w}")
                for r in range(r2):
                    t = get_tile("2", p2[0], idx2_t, xe_tab[:]); p2[0] += 128
                    nc.tensor.matmul(pre[:], diag_t[:, w, :], t,
                                     start=(r == 0), stop=False)
                nc.tensor.matmul(pre[:], XT_t[:, sl], MX_t[:], start=False, stop=False)
                nc.tensor.matmul(pre[:], X0T_t[:, sl], MX0_t[:], start=False, stop=False)
                nc.tensor.matmul(pre[:], auxv_t[:, sl], RC2_t[:], start=False, stop=True)
                relu_sb = wp.tile([128, 128], BF16, tag="relu", name=f"relu{w}")
                nc.scalar.activation(out=relu_sb[:], in_=pre[:],
                                     func=mybir.ActivationFunctionType.Relu)
                prt = psT.tile([128, 128], F32, tag="t", name=f"psRT{w}")
                nc.tensor.matmul(prt[:], relu_sb[:], Id_t[:], start=True, stop=True)
                rt_sb = wp.tile([128, 128], BF16, tag="rt", name=f"rt{w}")
                nc.scalar.copy(rt_sb[:], prt[:])
                pso = psF.tile([128, 128], F32, tag="fin", name=f"psO{w}")
                nc.tensor.matmul(pso[:], rt_sb[:], W3w2_t[:], start=True, stop=False)
                nc.tensor.matmul(pso[:], ones1_t[:], b2row_t[:], start=False, stop=True)
                o_sb = wp.tile([128, D], F32, tag="o_sb", name=f"o_sb{w}")
                nc.scalar.copy(o_sb[:], pso[:])
                nc.sync.dma_start(out_d[sl, :], o_sb[:])

    nc.finalize()
    return nc


def run(trace=False, nq=NQ, **inputs):
    in_maps, sched, unperm = prepare(inputs)
    nc = build(in_maps[0], sched, nq=nq)
    res = run_bass_kernel_spmd(nc, in_maps, list(range(C)), trace=trace)
    out = np.empty((N, D), np.float32)
    v_core, v_pos = unperm["v_core"], unperm["v_pos"]
    for c in range(C):
        oc = res.results[c]["out"]          # [VP, D]
        mask = v_core == c
        out[mask] = oc[v_pos[mask]]
    return out, res


def kernel(**inputs):
    """Harness entry point: full inputs in, full [N, D] float32 output."""
    out, _res = run(trace=False, **inputs)
    return out.astype(np.float32)


# revision 12
# speedup vs baseline: 1.6664x; 1.1805x over previous
"""Trainium2 Bass kernel for nn_MeanDegConv (gnn_message_passing) on 8 NeuronCores.

Round-based design: incidences are laid out as (window, round, slot) grids so
segment sums become PSUM-accumulating identity/diag matmuls (no per-tile
one-hot builds on the vector engine). Gather tables are bf16 (halved DMA
bytes) and gather descriptor generation rotates across SWDGE queues.

Self-contained: imports the Bass/Tile stack from /opt/trn_rl_repo (part of the
container environment) and hardcodes all shapes/sharding for the problem.
"""
import sys
for _p in ('/opt/trn_rl_repo',):
    if _p not in sys.path:
        sys.path.insert(0, _p)

import numpy as np

import concourse.bass as bass
import concourse.mybir as mybir
import concourse.tile as tile
import concourse.bacc as bacc
from concourse.bass_utils import run_bass_kernel_spmd

N, E, NNZ, D = 50000, 10000, 1000000, 128
C = 8
EPC, VPC = E // C, N // C          # 1250 edges, 6250 vertices per core
NWIN_E = (EPC + 127) // 128        # 10
NWIN_V = (VPC + 127) // 128        # 49
EP = NWIN_E * 128                  # 1280 padded edge slots per core
VP = NWIN_V * 128                  # 6272 padded vertex slots per core
CHUNK = 4096                       # gather indices per dma_gather call
TPC = CHUNK // 128                 # tiles per chunk
NQ = 4                             # SWDGE queues to rotate desc-gen across

SPLA = 32639                       # XA covers vertices [0, 32639); zero block after
XB_BASE = N - SPLA                 # 17361; XB covers [17361, 50000); zero block after
XTAB_ROWS = SPLA + 128             # 32767 rows per split table (128 zero rows)
XE_ROWS = C * EP                   # 10240 real xe rows
XE_ZERO = XE_ROWS                  # zero block start in xe_tab
XE_TAB_ROWS = XE_ROWS + 128        # 128 zero rows

F32 = mybir.dt.float32
BF16 = mybir.dt.bfloat16
I16 = mybir.dt.int16
BFNP = mybir.dt.np(BF16)


def _pack_idx16(idx32: np.ndarray) -> np.ndarray:
    """[L] int32 -> [128, L/16] int16 in the dma_gather wrap layout."""
    L = len(idx32)
    assert L % 16 == 0
    a = idx32.astype(np.int16).reshape(L // 16, 16).T  # [16, L/16]
    return np.ascontiguousarray(np.tile(a, (8, 1)))    # [128, L/16]


def _padlen(L):
    return ((L + CHUNK - 1) // CHUNK) * CHUNK


def prepare(inputs):
    X = np.asarray(inputs["X"], np.float32)
    X0 = np.asarray(inputs["X0"], np.float32)
    v = np.asarray(inputs["vertex"]).astype(np.int64)
    e = np.asarray(inputs["edges"]).astype(np.int64)
    W1_w = np.asarray(inputs["W1_w"], np.float32); W1_b = np.asarray(inputs["W1_b"], np.float32)
    W2_w = np.asarray(inputs["W2_w"], np.float32); W2_b = np.asarray(inputs["W2_b"], np.float32)
    W3_w1 = np.asarray(inputs["W3_w1"], np.float32); W3_b1 = np.asarray(inputs["W3_b1"], np.float32)
    W3_w2 = np.asarray(inputs["W3_w2"], np.float32); W3_b2 = np.asarray(inputs["W3_b2"], np.float32)

    deg_e = np.bincount(e, minlength=E)
    deg_v = np.bincount(v, minlength=N)

    # ---- folded weight matrices (float64 for accuracy, cast at the end)
    W2a = W2_w[:D].astype(np.float64); W2b1 = W2_w[D:2*D].astype(np.float64)
    w2b_log = W2_w[2*D].astype(np.float64)
    R1 = W3_w1[:D].astype(np.float64); R2 = W3_w1[D:2*D].astype(np.float64)
    R3 = W3_w1[2*D:3*D].astype(np.float64); r4 = W3_w1[3*D].astype(np.float64)
    W2bR = W2b1 @ R1
    K1 = (W1_w.astype(np.float64) @ W2bR).astype(np.float32)
    k2 = (w2b_log @ R1).astype(np.float32)
    c1 = (W1_b.astype(np.float64) @ W2bR).astype(np.float32)
    MX = (W2a @ R1 + R2).astype(np.float32)
    MX0 = R3.astype(np.float32)
    c0 = (W2_b.astype(np.float64) @ R1 + W3_b1).astype(np.float32)

    # ---- permutations: sort by degree desc, deal round-robin to cores
    eperm = np.argsort(-deg_e, kind="stable")
    e_core = np.empty(E, np.int64); e_pos = np.empty(E, np.int64)
    e_core[eperm] = np.arange(E) % C
    e_pos[eperm] = np.arange(E) // C
    vperm = np.argsort(-deg_v, kind="stable")
    v_core = np.empty(N, np.int64); v_pos = np.empty(N, np.int64)
    v_core[vperm] = np.arange(N) % C
    v_pos[vperm] = np.arange(N) // C

    # ---- stage 1: A/B balanced split per edge
    cls = np.where(v < XB_BASE, 0, np.where(v >= SPLA, 2, 1))
    nAf = np.bincount(e[cls == 0], minlength=E)
    nBf = np.bincount(e[cls == 2], minlength=E)
    cntA = np.clip((deg_e + 1) // 2, nAf, deg_e - nBf)

    cA = np.zeros((C, EP), np.int64); cB = np.zeros((C, EP), np.int64)
    cA[e_core, e_pos] = cntA
    cB[e_core, e_pos] = deg_e - cntA
    RA = cA.reshape(C, NWIN_E, 128).max(axis=(0, 2))
    RB = cB.reshape(C, NWIN_E, 128).max(axis=(0, 2))
    LA = int(RA.sum()) * 128
    LB = int(RB.sum()) * 128

    # order incidences by (edge, class): forced-A, middles, forced-B
    oinc = np.argsort(e * 4 + cls, kind="stable")
    e_s = e[oinc]; v_s = v[oinc]
    starts = np.searchsorted(e_s, np.arange(E))
    rank = np.arange(NNZ) - starts[e_s]
    isA = rank < cntA[e_s]
    ecore1 = e_core[e_s]; epos1 = e_pos[e_s]
    w1 = epos1 // 128; s1 = epos1 % 128
    offA = np.zeros(NWIN_E, np.int64); offA[1:] = np.cumsum(RA)[:-1]
    offB = np.zeros(NWIN_E, np.int64); offB[1:] = np.cumsum(RB)[:-1]
    posA = (offA[w1] + rank) * 128 + s1
    posB = (offB[w1] + (rank - cntA[e_s])) * 128 + s1

    # ---- stage 2 rounds
    cV = np.zeros((C, VP), np.int64)
    cV[v_core, v_pos] = deg_v
    R2r = cV.reshape(C, NWIN_V, 128).max(axis=(0, 2))
    L2 = int(R2r.sum()) * 128

    rowid_of_e = e_core * EP + e_pos
    o2 = np.argsort(v, kind="stable")
    v_s2 = v[o2]; e_s2 = e[o2]
    starts2 = np.searchsorted(v_s2, np.arange(N))
    rank2 = np.arange(NNZ) - starts2[v_s2]
    vcore2 = v_core[v_s2]; vpos2 = v_pos[v_s2]
    w2 = vpos2 // 128; s2w = vpos2 % 128
    off2 = np.zeros(NWIN_V, np.int64); off2[1:] = np.cumsum(R2r)[:-1]
    pos2 = (off2[w2] + rank2) * 128 + s2w
    val2 = rowid_of_e[e_s2]

    sched = {"RA": [int(x) for x in RA], "RB": [int(x) for x in RB],
             "R2": [int(x) for x in R2r],
             "LA": _padlen(LA), "LB": _padlen(LB), "L2": _padlen(L2)}

    # ---- shared consts
    Xb = X.astype(BFNP)
    XA = np.zeros((XTAB_ROWS, D), BFNP); XA[:SPLA] = Xb[:SPLA]
    XBt = np.zeros((XTAB_ROWS, D), BFNP); XBt[:N - XB_BASE] = Xb[XB_BASE:]
    deg_ef = deg_e.astype(np.float32); log_deg_e = np.log(deg_ef)
    deg_vf = deg_v.astype(np.float32); log_deg_v = np.log(deg_vf)

    consts = {
        "XA": XA, "XB": XBt,
        "iota": np.tile(np.arange(128, dtype=np.float32), (128, 1)).astype(BFNP),
        "iotacol": np.arange(128, dtype=np.float32).reshape(128, 1),
        "K1": K1.astype(BFNP),
        "K2": np.ascontiguousarray(np.stack([k2, c1])),
        "MX": MX, "MX0": MX0,
        "RC2": np.ascontiguousarray(np.stack([r4.astype(np.float32), c0])),
        "W3w2": W3_w2.astype(BFNP),
        "b2row": W3_b2.reshape(1, D),
        "ones1": np.ones((1, 128), np.float32),
    }

    edge_at = np.full((C, EP), -1, np.int64)
    edge_at[e_core, e_pos] = np.arange(E)
    vert_at = np.full((C, VP), -1, np.int64)
    vert_at[v_core, v_pos] = np.arange(N)

    in_maps = []
    for c in range(C):
        mA = (ecore1 == c) & isA
        mB = (ecore1 == c) & (~isA)
        sA = SPLA + (np.arange(sched["LA"], dtype=np.int32) % 128)
        sA[posA[mA]] = v_s[mA]
        sB = SPLA + (np.arange(sched["LB"], dtype=np.int32) % 128)
        sB[posB[mB]] = v_s[mB] - XB_BASE

        m2 = vcore2 == c
        s2 = XE_ZERO + (np.arange(sched["L2"], dtype=np.int32) % 128)
        s2[pos2[m2]] = val2[m2]

        de = np.ones(EP, np.float32); le = np.zeros(EP, np.float32)
        msk = edge_at[c] >= 0
        de[msk] = deg_ef[edge_at[c][msk]]
        le[msk] = log_deg_e[edge_at[c][msk]]
        auxe = np.ascontiguousarray(np.stack([de * le, de]))
        invde_col = np.ascontiguousarray((1.0 / de).reshape(NWIN_E, 128).T)

        dv = np.ones(VP, np.float32); lv = np.zeros(VP, np.float32)
        vm = vert_at[c] >= 0
        dv[vm] = deg_vf[vert_at[c][vm]]
        lv[vm] = log_deg_v[vert_at[c][vm]]
        auxv = np.ascontiguousarray(np.stack([lv, np.ones(VP, np.float32)]))
        invdv_col = np.ascontiguousarray((1.0 / dv).reshape(NWIN_V, 128).T)

        Xp = np.zeros((VP, D), np.float32); X0p = np.zeros((VP, D), np.float32)
        Xp[vm] = X[vert_at[c][vm]]
        X0p[vm] = X0[vert_at[c][vm]]

        m = dict(consts)
        m.update({
            "idxA": _pack_idx16(sA), "idxB": _pack_idx16(sB),
            "idx2": _pack_idx16(s2),
            "auxe": auxe, "invde_col": invde_col,
            "auxv": auxv, "invdv_col": invdv_col,
            "XT": np.ascontiguousarray(Xp.T), "X0T": np.ascontiguousarray(X0p.T),
        })
        in_maps.append(m)
    unperm = {"v_core": v_core, "v_pos": v_pos}
    return in_maps, sched, unperm


def build(in_map0, sched, nq=NQ):
    RA, RB, R2 = sched["RA"], sched["RB"], sched["R2"]
    nc = bacc.Bacc(None, num_swdge_queues=nq, dynamic_dma_scratch_size=32768)

    def param(name, dt=F32):
        arr = in_map0[name]
        return nc.declare_dram_parameter(name, list(arr.shape), dt, isOutput=False)

    XA_d = param("XA", BF16); XB_d = param("XB", BF16)
    iota_d = param("iota", BF16); iotacol_d = param("iotacol")
    K1_d = param("K1", BF16); K2_d = param("K2")
    MX_d = param("MX"); MX0_d = param("MX0"); RC2_d = param("RC2")
    W3w2_d = param("W3w2", BF16); b2row_d = param("b2row"); ones1_d = param("ones1")
    idxA_d = param("idxA", I16); idxB_d = param("idxB", I16); idx2_d = param("idx2", I16)
    auxe_d = param("auxe"); invde_d = param("invde_col")
    auxv_d = param("auxv"); invdv_d = param("invdv_col")
    XT_d = param("XT"); X0T_d = param("X0T")
    out_d = nc.declare_dram_parameter("out", [VP, D], F32, isOutput=True)

    # Rotate desc-gen across SWDGE queues 1..3: their Q7 core-pairs generate
    # descriptors off the Pool engine's critical path (queue 0 blocks ~32us).
    qrot = [1, 2, 3, 0] if nq == 4 else list(range(nq))
    qctr = [0]

    def next_q():
        q = qrot[qctr[0] % len(qrot)]
        qctr[0] += 1
        return q

    with tile.TileContext(nc) as tc:
        with (
            tc.tile_pool(name="const", bufs=1) as cp,
            tc.tile_pool(name="stream", bufs=1) as sp,
            tc.tile_pool(name="g", bufs=6) as gp,
            tc.tile_pool(name="work", bufs=3) as wp,
            tc.tile_pool(name="psA", bufs=2, space="PSUM") as psA,
            tc.tile_pool(name="psT", bufs=2, space="PSUM") as psT,
            tc.tile_pool(name="psF", bufs=2, space="PSUM") as psF,
            tc.tile_pool(name="dram", bufs=1, space="DRAM") as dp,
        ):
            def load(pool, dram_ap, name, dt=F32, eng=None):
                t = pool.tile(list(dram_ap.shape), dt, name=name, tag=name)
                (eng or nc.sync).dma_start(t[:], dram_ap[:])
                return t

            idxA_t = load(sp, idxA_d, "idxA", I16)
            idxB_t = load(sp, idxB_d, "idxB", I16)
            idx2_t = load(sp, idx2_d, "idx2", I16)
            iota_t = load(cp, iota_d, "iota", BF16)
            iotacol_t = load(cp, iotacol_d, "iotacol")
            K1_t = load(cp, K1_d, "K1", BF16); K2_t = load(cp, K2_d, "K2")
            MX_t = load(cp, MX_d, "MX"); MX0_t = load(cp, MX0_d, "MX0")
            RC2_t = load(cp, RC2_d, "RC2")
            W3w2_t = load(cp, W3w2_d, "W3w2", BF16)
            b2row_t = load(cp, b2row_d, "b2row"); ones1_t = load(cp, ones1_d, "ones1")
            auxe_t = load(cp, auxe_d, "auxe"); invde_t = load(cp, invde_d, "invde")
            auxv_t = load(cp, auxv_d, "auxv"); invdv_t = load(cp, invdv_d, "invdv")
            XT_t = load(cp, XT_d, "XT", eng=nc.scalar)
            X0T_t = load(cp, X0T_d, "X0T", eng=nc.scalar)

            # identity (bf16): Id[s, j] = (iota[s, j] == s)
            Id_t = cp.tile([128, 128], BF16, name="Id", tag="Id")
            nc.vector.tensor_scalar(
                out=Id_t[:], in0=iota_t[:], scalar1=iotacol_t[:, 0:1],
                scalar2=None, op0=mybir.AluOpType.is_equal)
            # per-vertex-window diag(1/deg_v) bf16
            diag_t = cp.tile([128, NWIN_V, 128], BF16, name="diag", tag="diag")
            for w in range(NWIN_V):
                nc.vector.tensor_scalar(
                    out=diag_t[:, w, :], in0=iota_t[:], scalar1=iotacol_t[:, 0:1],
                    scalar2=invdv_t[:, w:w + 1], op0=mybir.AluOpType.is_equal,
                    op1=mybir.AluOpType.mult)

            H = (NWIN_E // 2) * 128
            xe_lo = dp.tile([H, D], BF16)
            xe_hi = dp.tile([EP - H, D], BF16)
            xe_all_lo = dp.tile([C * (NWIN_E // 2) * 128, D], BF16, addr_space="Shared")
            xe_all_hi = dp.tile([C * (EP - (NWIN_E // 2) * 128), D], BF16, addr_space="Shared")
            xe_tab = dp.tile([XE_TAB_ROWS, D], BF16)
            zrow = wp.tile([128, D], BF16, tag="zrow", name="zrow")
            nc.vector.memset(zrow[:], 0.0)
            nc.sync.dma_start(xe_tab[XE_ZERO:XE_ZERO + 128, :], zrow[:])

            # ---- lazy chunked gathers: one shared SBUF ring, issue on demand
            chunks = {}
            used = {"A": sum(RA) * 128, "B": sum(RB) * 128, "2": sum(R2) * 128}

            def get_tile(stream, pos, idx_t, in_ap):
                lst = chunks.setdefault(stream, [])
                ci = pos // CHUNK
                while len(lst) <= ci:
                    k = len(lst)
                    rem = min(CHUNK, used[stream] - k * CHUNK)
                    g = gp.tile([128, TPC, D], BF16, tag="g", name=f"g{stream}{k}")
                    nc.gpsimd.dma_gather(
                        out_ap=g[:, :rem // 128, :], in_ap=in_ap,
                        idxs_ap=idx_t[:, k * (CHUNK // 16):
                                      k * (CHUNK // 16) + rem // 16],
                        num_idxs=rem, num_idxs_reg=rem,
                        single_packet=False, elem_size=D, queue_num=next_q())
                    lst.append(g)
                return lst[ci][:, (pos % CHUNK) // 128, :]

            # ============ stage 1 ============
            pA = [0]; pB = [0]
            for w in range(NWIN_E):
                ra, rb = RA[w], RB[w]
                ps = psA.tile([128, 128], F32, tag="acc", name=f"psS{w}")
                for r in range(ra):
                    t = get_tile("A", pA[0], idxA_t, XA_d[:]); pA[0] += 128
                    nc.tensor.matmul(ps[:], Id_t[:], t, start=(r == 0), stop=False)
                for r in range(rb):
                    t = get_tile("B", pB[0], idxB_t, XB_d[:]); pB[0] += 128
                    nc.tensor.matmul(ps[:], Id_t[:], t,
                                     start=False, stop=(r == rb - 1))
                s_sb = wp.tile([128, 128], BF16, tag="s_sb", name=f"s_sb{w}")
                nc.scalar.copy(s_sb[:], ps[:])
                pst = psT.tile([128, 128], F32, tag="t", name=f"psT{w}")
                nc.tensor.matmul(pst[:], s_sb[:], Id_t[:], start=True, stop=True)
                st_sb = wp.tile([128, 128], BF16, tag="st_sb", name=f"st_sb{w}")
                nc.scalar.copy(st_sb[:], pst[:])
                pxe = psF.tile([128, 128], F32, tag="fin", name=f"psXE{w}")
                nc.tensor.matmul(pxe[:], st_sb[:], K1_t[:], start=True, stop=False)
                nc.tensor.matmul(pxe[:], auxe_t[:, w * 128:(w + 1) * 128], K2_t[:],
                                 start=False, stop=True)
                xe_sb = wp.tile([128, D], BF16, tag="xe_sb", name=f"xe_sb{w}")
                nc.scalar.activation(
                    out=xe_sb[:], in_=pxe[:],
                    func=mybir.ActivationFunctionType.Copy,
                    scale=invde_t[:, w:w + 1])
                if w * 128 < H:
                    nc.sync.dma_start(xe_lo[w * 128:(w + 1) * 128, :], xe_sb[:])
                else:
                    nc.sync.dma_start(xe_hi[w * 128 - H:(w + 1) * 128 - H, :], xe_sb[:])

            # ============ allgather (two halves, first overlaps stage-1 tail)
            nc.gpsimd.collective_compute(
                "AllGather", mybir.AluOpType.bypass,
                replica_groups=[list(range(C))],
                ins=[xe_lo.opt()], outs=[xe_all_lo.opt()])
            nc.gpsimd.collective_compute(
                "AllGather", mybir.AluOpType.bypass,
                replica_groups=[list(range(C))],
                ins=[xe_hi.opt()], outs=[xe_all_hi.opt()])
            for cc in range(C):
                nc.sync.dma_start(xe_tab[cc * EP: cc * EP + H, :],
                                  xe_all_lo[cc * H:(cc + 1) * H, :])
                nc.sync.dma_start(xe_tab[cc * EP + H:(cc + 1) * EP, :],
                                  xe_all_hi[cc * (EP - H):(cc + 1) * (EP - H), :])

            # ============ stage 2 ============
            p2 = [0]
            for w in range(NWIN_V):
                sl = slice(w * 128, (w + 1) * 128)
                r2 = R2[w]
                pre = psA.tile([128, 128], F32, tag="acc", name=f"psP{w}")
                for r in range(r2):
                    t = get_tile("2", p2[0], idx2_t, xe_tab[:]); p2[0] += 128
                    nc.tensor.matmul(pre[:], diag_t[:, w, :], t,
                                     start=(r == 0), stop=False)
                nc.tensor.matmul(pre[:], XT_t[:, sl], MX_t[:], start=False, stop=False)
                nc.tensor.matmul(pre[:], X0T_t[:, sl], MX0_t[:], start=False, stop=False)
                nc.tensor.matmul(pre[:], auxv_t[:, sl], RC2_t[:], start=False, stop=True)
                relu_sb = wp.tile([128, 128], BF16, tag="relu", name=f"relu{w}")
                nc.scalar.activation(out=relu_sb[:], in_=pre[:],
                                     func=mybir.ActivationFunctionType.Relu)
                prt = psT.tile([128, 128], F32, tag="t", name=f"psRT{w}")
                nc.tensor.matmul(prt[:], relu_sb[:], Id_t[:], start=True, stop=True)
                rt_sb = wp.tile([128, 128], BF16, tag="rt", name=f"rt{w}")
                nc.scalar.copy(rt_sb[:], prt[:])
                pso = psF.tile([128, 128], F32, tag="fin", name=f"psO{w}")
                nc.tensor.matmul(pso[:], rt_sb[:], W3w2_t[:], start=True, stop=False)
                nc.tensor.matmul(pso[:], ones1_t[:], b2row_t[:], start=False, stop=True)
                o_sb = wp.tile([128, D], F32, tag="o_sb", name=f"o_sb{w}")
                nc.scalar.copy(o_sb[:], pso[:])
                nc.sync.dma_start(out_d[sl, :], o_sb[:])

    nc.finalize()
    return nc


def run(trace=False, nq=NQ, **inputs):
    in_maps, sched, unperm = prepare(inputs)
    nc = build(in_maps[0], sched, nq=nq)
    res = run_bass_kernel_spmd(nc, in_maps, list(range(C)), trace=trace)
    out = np.empty((N, D), np.float32)
    v_core, v_pos = unperm["v_core"], unperm["v_pos"]
    for c in range(C):
        oc = res.results[c]["out"]          # [VP, D]
        mask = v_core == c
        out[mask] = oc[v_pos[mask]]
    return out, res


def kernel(**inputs):
    """Harness entry point: full inputs in, full [N, D] float32 output."""
    out, _res = run(trace=False, **inputs)
    return out.astype(np.float32)
